# revision 8
# baseline (speedup 1.0000x reference)
"""ChebConv GNN (K=6, N=200000, E=3200000, F=64) on 8 trn2 NeuronCores.

Strategy:
  - Nodes are row-sharded across 8 cores (25000 dst nodes/core, padded to
    196*128=25088). Edges (incl. self loops) are assigned to the core owning
    their destination, sorted by destination, and grouped into 128-row dst
    blocks; each block's edges are padded to chunks of 128.
  - prop(z) = segment_sum(norm * z[src], dst) becomes, per dst block:
      gather z[src] for all the block's edges via one indirect DMA, then for
      each 128-edge chunk build a weighted one-hot matrix
      oh[e, d] = (d == dstlocal[e]) * w[e] with a single dual-op
      tensor_scalar, and accumulate matmul(lhsT=gathered, rhs=oh) into PSUM.
      The PSUM result is the feature-major block of S_w @ z.
  - The Chebyshev recursion/output algebra is folded on the host:
      B_k := S_w T_{k-1} (raw per-prop result, single weight set w),
      T_k = 2 B_k - T_{k-2} (the x2 rides a scaled identity in the PE
      transpose used to produce the node-major z for the next prop),
      out = x Vx + sum_k B_k V_k with host-precomputed V matrices.
  - Between props, the new T_k rows are AllGathered (6.4MB/rank) so every
    core can gather arbitrary source rows next prop.
  - Output is returned feature-major/block-packed and unpacked on the host.
"""

import sys
import numpy as np

for _p in ("/opt/trn_rl_repo",):
    if _p not in sys.path:
        sys.path.insert(0, _p)

P = 128
F = 64
KCHEB = 6
NCORES = 8


def preprocess(x, LBO_index, LBO_weight, weight, bias, n_cores=NCORES):
    """Host-side graph preprocessing. Returns (in_maps, cfg)."""
    N = x.shape[0]
    assert N % n_cores == 0
    npc = N // n_cores
    nblk = (npc + P - 1) // P
    npc_pad = nblk * P
    Npad = n_cores * npc_pad

    x = np.ascontiguousarray(np.asarray(x, dtype=np.float32))
    idx = np.asarray(LBO_index)
    w = 2.0 * np.asarray(LBO_weight, dtype=np.float32) / np.float32(-2.0)
    w = np.where(np.isinf(w), np.float32(0.0), w).astype(np.float32)

    loops = np.arange(N, dtype=np.int64)
    src = np.concatenate([idx[0].astype(np.int64), loops])
    dst = np.concatenate([idx[1].astype(np.int64), loops])
    wt = np.concatenate([w, np.full(N, -1.0, dtype=np.float32)])

    order = np.argsort(dst, kind="stable")
    src, dst, wt = src[order], dst[order], wt[order]
    # remap source indices into the padded node layout
    src_pad = (src // npc) * npc_pad + (src % npc)

    core_bounds = np.searchsorted(dst, np.arange(n_cores + 1) * npc)

    # per-(core, block) chunk counts; padded to the max over cores per block
    per_core = []
    Cb = np.ones(nblk, dtype=np.int64)
    for c in range(n_cores):
        s, e = int(core_bounds[c]), int(core_bounds[c + 1])
        dl = dst[s:e] - c * npc
        blk = dl // P
        cnt = np.bincount(blk, minlength=nblk)
        per_core.append((s, e, dl, blk, cnt))
        Cb = np.maximum(Cb, (cnt + P - 1) // P)
    off_b = np.concatenate([[0], np.cumsum(Cb)]).astype(np.int64)
    totC = int(off_b[-1])

    # padded full-x gather table (node-major, padded layout)
    xf_p = np.zeros((Npad, F), dtype=np.float32)
    xf_p.reshape(n_cores, npc_pad, F)[:, :npc] = x.reshape(n_cores, npc, F)

    # weight combinations
    W = np.asarray(weight, dtype=np.float32)
    V = np.empty((6, F, F), dtype=np.float32)
    V[0] = W[0] - W[2] + W[4]            # Vx
    V[1] = W[1] - W[3] + W[5]            # V1
    V[2] = 2.0 * (W[2] - W[4])
    V[3] = 2.0 * (W[3] - W[5])
    V[4] = 2.0 * W[4]
    V[5] = 2.0 * W[5]
    vw = np.ascontiguousarray(V.transpose(1, 0, 2).reshape(F, 6 * F))

    cid = np.tile(np.arange(P, dtype=np.float32), (P, 1))
    idm = np.zeros((F, 2 * F), dtype=np.float32)
    idm[:, :F] = np.eye(F, dtype=np.float32)
    idm[:, F:] = 2.0 * np.eye(F, dtype=np.float32)
    bia = np.tile(np.asarray(bias, dtype=np.float32), 2)[:, None].copy()

    in_maps = []
    for c in range(n_cores):
        s, e, dl, blk, cnt = per_core[c]
        ne = e - s
        blk_starts = np.concatenate([[0], np.cumsum(cnt)]).astype(np.int64)
        j_in_block = np.arange(ne, dtype=np.int64) - blk_starts[blk]
        pp = j_in_block % P
        cc = j_in_block // P
        col = off_b[blk] + cc
        offs = np.zeros((P, totC), dtype=np.int32)  # pad -> row 0 (w=0)
        meta = np.zeros((P, 2 * totC), dtype=np.float32)
        offs[pp, col] = src_pad[s:e].astype(np.int32)
        meta[pp, 2 * col] = (dl - blk * P).astype(np.float32)
        meta[pp, 2 * col + 1] = wt[s:e]

        xown = np.zeros((npc_pad, F), dtype=np.float32)
        xown[:npc] = x[c * npc:(c + 1) * npc]
        xfm = np.ascontiguousarray(xown.T)

        in_maps.append({
            "xf": xf_p,
            "xown": xown,
            "xfm": xfm,
            "offs": offs,
            "meta": meta,
            "vw": vw,
            "cid": cid,
            "idm": idm,
            "bia": bia,
        })

    cfg = dict(N=N, Npad=Npad, npc=npc, npc_pad=npc_pad, nblk=nblk,
               totC=totC, Cb=[int(c) for c in Cb],
               off_b=[int(o) for o in off_b], n_cores=n_cores)
    return in_maps, cfg


def build_program(cfg, enable_asserts=False):
    """Build the SPMD bass program (identical on every core)."""
    from contextlib import ExitStack
    import concourse.bass as bass
    import concourse.bacc as bacc
    import concourse.mybir as mybir
    import concourse.tile as tile

    dt = mybir.dt
    n_cores = cfg["n_cores"]
    Npad, npc_pad, nblk = cfg["Npad"], cfg["npc_pad"], cfg["nblk"]
    totC, Cb, off_b = cfg["totC"], cfg["Cb"], cfg["off_b"]
    npair = (nblk + 1) // 2

    nc = bacc.Bacc(
        "TRN2",
        target_bir_lowering=False,
        debug=False,
        enable_asserts=enable_asserts,
        num_devices=n_cores,
    )

    xf = nc.dram_tensor("xf", [Npad, F], dt.float32, kind="ExternalInput").ap()
    xown = nc.dram_tensor("xown", [npc_pad, F], dt.float32, kind="ExternalInput").ap()
    xfm = nc.dram_tensor("xfm", [F, npc_pad], dt.float32, kind="ExternalInput").ap()
    offs = nc.dram_tensor("offs", [P, totC], dt.int32, kind="ExternalInput").ap()
    meta = nc.dram_tensor("meta", [P, 2 * totC], dt.float32, kind="ExternalInput").ap()
    vw = nc.dram_tensor("vw", [F, 6 * F], dt.float32, kind="ExternalInput").ap()
    cid = nc.dram_tensor("cid", [P, P], dt.float32, kind="ExternalInput").ap()
    idm = nc.dram_tensor("idm", [F, 2 * F], dt.float32, kind="ExternalInput").ap()
    bia = nc.dram_tensor("bia", [P, 1], dt.float32, kind="ExternalInput").ap()
    outp = nc.dram_tensor("outp", [P, npair * P], dt.float32, kind="ExternalOutput").ap()

    AO = mybir.AluOpType

    with tile.TileContext(nc) as tc, ExitStack() as ctx:
        dram = ctx.enter_context(tc.tile_pool(name="dram", bufs=1, space="DRAM"))
        ag = [dram.tile([npc_pad, F], dt.float32, name=f"agin{k}")
              for k in range(1, 5)]
        z_addr = "Shared" if n_cores > 4 else "Local"
        zk = [dram.tile([Npad, F], dt.float32, name=f"z{k}", addr_space=z_addr)
              for k in range(1, 5)]

        cons = ctx.enter_context(tc.tile_pool(name="cons", bufs=1))
        cid_sb = cons.tile([P, P], dt.float32, name="cid_sb")
        id_sb = cons.tile([F, 2 * F], dt.float32, name="id_sb")
        vw_sb = cons.tile([F, 6 * F], dt.float32, name="vw_sb")
        bia_sb = cons.tile([P, 1], dt.float32, name="bia_sb")
        out_acc = cons.tile([P, npair * P], dt.float32, name="out_acc")
        nc.sync.dma_start(out=cid_sb[:, :], in_=cid)
        nc.sync.dma_start(out=id_sb[:, :], in_=idm)
        nc.sync.dma_start(out=vw_sb[:, :], in_=vw)
        nc.sync.dma_start(out=bia_sb[:, :], in_=bia)
        # touch constants on DVE once so later TensorScalarPtr ops (1 sync-wait
        # slot in walrus codegen) don't need to wait on these DMAs
        scr = cons.tile([P, 2], dt.float32, name="scr")
        nc.vector.tensor_copy(out=scr[:, 0:1], in_=cid_sb[:, 0:1])
        nc.vector.tensor_copy(out=scr[:, 1:2], in_=bia_sb[:, 0:1])

        Cmax = max(Cb)
        gp = ctx.enter_context(tc.tile_pool(name="gp", bufs=4))
        mp = ctx.enter_context(tc.tile_pool(name="mp", bufs=4))
        ohp = ctx.enter_context(tc.tile_pool(name="ohp", bufs=8))
        smp = ctx.enter_context(tc.tile_pool(name="smp", bufs=4))
        pap = ctx.enter_context(tc.tile_pool(name="pap", bufs=2, space="PSUM"))
        ptp = ctx.enter_context(tc.tile_pool(name="ptp", bufs=2, space="PSUM"))
        pwp = ctx.enter_context(tc.tile_pool(name="pwp", bufs=2, space="PSUM"))

        for k in range(1, 6):
            table = xf if k == 1 else zk[k - 2][:, :]
            prev = None
            if k == 2:
                prev = xown
            elif k in (3, 4):
                prev = ag[k - 3][:, :]

            for b in range(nblk):
                C = Cb[b]
                o0 = off_b[b]
                offs_sb = mp.tile([P, Cmax], dt.int32, tag="offs", name="offs_sb")
                m_sb = mp.tile([P, 2 * Cmax], dt.float32, tag="m", name="m_sb")
                nc.sync.dma_start(out=offs_sb[:, :C], in_=offs[:, o0:o0 + C])
                nc.sync.dma_start(out=m_sb[:, :2 * C], in_=meta[:, 2 * o0:2 * (o0 + C)])

                # one indirect DMA per 128-edge chunk: HW semantics are one
                # index per partition, 64 contiguous elements (one row) each
                g = gp.tile([P, Cmax * F], dt.float32, tag="g", name="g")
                for j in range(C):
                    nc.gpsimd.indirect_dma_start(
                        out=g[:, j * F:(j + 1) * F],
                        out_offset=None,
                        in_=table,
                        in_offset=bass.IndirectOffsetOnAxis(
                            ap=offs_sb[:, j:j + 1], axis=0),
                    )

                # absorb the m_sb DMA wait on DVE before the one-hot ops
                # (TensorScalarPtr has a single sync-wait slot)
                mtch = smp.tile([P, 1], dt.float32, tag="mtch", name="mtch")
                nc.vector.tensor_copy(out=mtch[:, :], in_=m_sb[:, 0:1])

                pa = pap.tile([F, P], dt.float32, tag="pa", name="pa")
                for j in range(C):
                    oh = ohp.tile([P, P], dt.float32, tag="oh", name="oh")
                    nc.vector.tensor_scalar(
                        out=oh[:, :],
                        in0=cid_sb[:, :],
                        scalar1=m_sb[:, 2 * j:2 * j + 1],
                        scalar2=m_sb[:, 2 * j + 1:2 * j + 2],
                        op0=AO.is_equal,
                        op1=AO.mult,
                    )
                    nc.tensor.matmul(
                        out=pa[:, :],
                        lhsT=g[:, j * F:(j + 1) * F],
                        rhs=oh[:, :],
                        start=(j == 0),
                        stop=(j == C - 1),
                    )

                b_fm = smp.tile([F, P], dt.float32, tag="bfm", name="b_fm")
                nc.scalar.copy(out=b_fm[:, :], in_=pa[:, :])

                pw = pwp.tile([F, P], dt.float32, tag="pw", name="pw")
                nc.tensor.matmul(
                    out=pw[:, :],
                    lhsT=vw_sb[:, k * F:(k + 1) * F],
                    rhs=b_fm[:, :],
                    start=True,
                    stop=(k != 1),
                )
                if k == 1:
                    xb_fm = smp.tile([F, P], dt.float32, tag="xbfm", name="xb_fm")
                    nc.sync.dma_start(out=xb_fm[:, :], in_=xfm[:, b * P:(b + 1) * P])
                    nc.tensor.matmul(
                        out=pw[:, :],
                        lhsT=vw_sb[:, 0:F],
                        rhs=xb_fm[:, :],
                        start=False,
                        stop=True,
                    )

                par, i = b & 1, b >> 1
                oa = out_acc[F * par:F * (par + 1), i * P:(i + 1) * P]
                if k == 1:
                    nc.vector.tensor_copy(out=oa, in_=pw[:, :])
                else:
                    nc.vector.tensor_tensor(out=oa, in0=oa, in1=pw[:, :], op=AO.add)

                if k <= 4:
                    pt = ptp.tile([P, F], dt.float32, tag="pt", name="pt")
                    ident = id_sb[:, 0:F] if k == 1 else id_sb[:, F:2 * F]
                    # regular matmul: pt = b_fm.T @ (s*I) — transpose-mode would
                    # reject the scaled identity
                    nc.tensor.matmul(
                        out=pt[:, :], lhsT=b_fm[:, :], rhs=ident,
                        start=True, stop=True,
                    )
                    t_nm = smp.tile([P, F], dt.float32, tag="tnm", name="t_nm")
                    if k == 1:
                        nc.vector.tensor_copy(out=t_nm[:, :], in_=pt[:, :])
                    else:
                        pb = smp.tile([P, F], dt.float32, tag="pb", name="pb")
                        nc.sync.dma_start(out=pb[:, :], in_=prev[b * P:(b + 1) * P, :])
                        nc.vector.tensor_tensor(
                            out=t_nm[:, :], in0=pt[:, :], in1=pb[:, :], op=AO.subtract
                        )
                    nc.sync.dma_start(
                        out=ag[k - 1][b * P:(b + 1) * P, :], in_=t_nm[:, :]
                    )

            if k <= 4:
                nc.gpsimd.collective_compute(
                    "AllGather",
                    AO.bypass,
                    replica_groups=[list(range(n_cores))],
                    ins=[ag[k - 1][:, :]],
                    outs=[zk[k - 1][:, :]],
                )

        # bias + store (feature-major packed; host unpacks)
        nchunk = 4
        csz = (npair * P + nchunk - 1) // nchunk
        for t in range(nchunk):
            lo = t * csz
            hi = min((t + 1) * csz, npair * P)
            if lo >= hi:
                continue
            nc.vector.tensor_scalar(
                out=out_acc[:, lo:hi],
                in0=out_acc[:, lo:hi],
                scalar1=bia_sb[:, 0:1],
                scalar2=None,
                op0=AO.add,
            )
            nc.sync.dma_start(out=outp[:, lo:hi], in_=out_acc[:, lo:hi])

    nc.compile()
    return nc


def unpack_output(results, cfg):
    """Assemble the full [N, F] output from per-core packed fm outputs."""
    N, npc, nblk = cfg["N"], cfg["npc"], cfg["nblk"]
    n_cores = cfg["n_cores"]
    out = np.empty((N, F), dtype=np.float32)
    for c in range(n_cores):
        packed = results[c]["outp"]  # [P, npair*P]
        for b in range(nblk):
            par, i = b & 1, b >> 1
            rows = min(P, npc - b * P)
            blkv = packed[F * par:F * (par + 1), i * P:i * P + rows]  # [F, rows]
            out[c * npc + b * P: c * npc + b * P + rows] = blkv.T
    return out


_CACHE = {}


def kernel(x, LBO_index, LBO_weight, weight, bias):
    from concourse import bass_utils

    x = np.asarray(x)
    in_maps, cfg = preprocess(x, LBO_index, LBO_weight, weight, bias)
    key = (x.shape, cfg["totC"])
    if key not in _CACHE:
        _CACHE[key] = build_program(cfg)
    nc = _CACHE[key]
    res = bass_utils.run_bass_kernel_spmd(
        nc, in_maps, core_ids=list(range(cfg["n_cores"]))
    )
    return unpack_output(res.results, cfg)


if __name__ == "__main__":
    import reference

    inputs = reference.setup_inputs()
    inputs = {k: np.asarray(v) for k, v in inputs.items()}
    got = kernel(**inputs)
    exp = np.asarray(reference.reference(**inputs))
    err = np.abs(got - exp).max() / (np.abs(exp).max() + 1e-30)
    print("Relative error:", err)
